# revision 29
# baseline (speedup 1.0000x reference)
"""Trainium2 Bass kernel for nn_MLP_Interpolate.

Reference computation (out_size=512, H=W=128, exact 4x nearest upsample):
  out[b, :, 4k+r, 4l+s] = relu(x[b,:,k,l] @ W1[:64] + c[r,s]) @ W2 + b2
  c[r,s] = rel_y(r)*W1[64] + rel_x(s)*W1[65] + b1,  rel(t) = (2t-3)/4

v2 design (8 cores, shard = (batch, H-half), 64 input rows/core):
  - F = W1c^T x on PE in f32r (1 col/cycle at 512-col moving dim) with a
    block-diagonal stationary: two 64-channel pixel groups per pass.
  - F copied once to SBUF as bf16; the 16 bias+relu variants write bf16
    h tiles, split across DVE (SBUF src, 2x/4x perf modes), ACT (PSUM
    src), and GPSIMD (SBUF src; GPSIMD cannot touch PSUM).
  - pred = h @ W2diag on PE in bf16 (1 col/cycle), 8 matmuls per
    (t, r) into one PSUM tile at quadrant bases {0,32,64,96}.
  - One wide PSUM->SBUF stage copy per (t, r), then ONE DMA per (t, r)
    using a partition-split access pattern that scatters rows directly
    into the [3, 256, 512] output block.
"""

import numpy as np
import ml_dtypes

import concourse.bass as bass
import concourse.bacc as bacc
import concourse.mybir as mybir
import concourse.tile as tile
from concourse.bass_utils import run_bass_kernel_spmd

# Problem constants (hardcoded per contract)
B, C, H, W = 4, 64, 128, 128
OUT = 512
NF = 64
N_CORES = 8
RPC = H // 2       # 64 input rows per core
NT = RPC // 16     # 4 F-tiles, 16 input rows each (2 groups of 8)
REL = np.array([-0.75, -0.25, 0.25, 0.75], dtype=np.float32)

fp32 = mybir.dt.float32
bf16 = mybir.dt.bfloat16
f32r = mybir.dt.float32r

# Engine split, measured on HW: DVE bf16 SBUF->SBUF tensor_scalar hits
# 4x mode (~400-480ns/1024el) -> DVE takes all 16 relu variants. ACT is
# always 1x but PSUM reads are cheap -> ACT takes every PSUM->SBUF copy
# (4 stage copies + 2 F-half copies per tile). GPSIMD tensor ops run
# ~25x below roofline (14.7us/op) and are used only to issue DMAs.
RELU_ENG = (
    ("dve", "dve", "dve", "dve"),
    ("dve", "dve", "dve", "dve"),
    ("dve", "dve", "dve", "dve"),
    ("dve", "dve", "dve", "dve"),
)
# stage-copy engine per r
STAGE_ENG = ("act", "act", "act", "act")

_CACHE = {}


def _build_program():
    if "nc" in _CACHE:
        return _CACHE["nc"]

    nc = bacc.Bacc("TRN2", target_bir_lowering=False, debug=False,
                   num_devices=N_CORES)

    x_d = nc.dram_tensor("x", [C, RPC, W], bf16, kind="ExternalInput")
    w1_d = nc.dram_tensor("w1diag", [128, 128], bf16, kind="ExternalInput")
    w2_d = nc.dram_tensor("w2diag", [128, 6], bf16, kind="ExternalInput")
    crs_d = nc.dram_tensor("crsT", [128, 16], fp32, kind="ExternalInput")
    # raw dump of the staged pred tiles; descrambled host-side in _gather
    # (DMA APs are limited to 3 dims, so a direct scatter into the
    # [3, 256, 512] layout is not expressible -- dump + host reorder is).
    # One 48KB DMA per (t, quadrant): 6 useful partitions x 4 r-phases.
    out_d = nc.dram_tensor("out", [NT, 4, 6, 4, 2 * OUT], bf16,
                           kind="ExternalOutput")

    with tile.TileContext(nc) as tc:
        with (
            tc.tile_pool(name="consts", bufs=1) as consts,
            tc.tile_pool(name="xin", bufs=2) as xin,
            tc.tile_pool(name="fsb", bufs=2) as fsbp,
            tc.tile_pool(name="hbuf", bufs=8) as hbuf,
            tc.tile_pool(name="stage", bufs=2) as stagep,
            tc.tile_pool(name="fpsum", bufs=2, space=bass.MemorySpace.PSUM) as fpsum,
            tc.tile_pool(name="ppsum", bufs=2, space=bass.MemorySpace.PSUM) as ppsum,
        ):
            w1_sb = consts.tile([128, 128], bf16)
            w2_sb = consts.tile([128, 6], bf16)
            crs_sb = consts.tile([128, 16], fp32)
            nc.sync.dma_start(w1_sb[:], w1_d[:])
            nc.sync.dma_start(w2_sb[:], w2_d[:])
            nc.sync.dma_start(crs_sb[:], crs_d[:])

            x_tiles = {}
            f_tiles = {}
            fs_tiles = {}

            def load_x(t):
                # split per 64-channel group across two issuing queues so
                # the transfers land on different DMA engines (~45GB/s per
                # engine; a single 512KB DMA serializes for >10us)
                xt = xin.tile([128, 8, W], bf16, tag="xt")
                nc.scalar.dma_start(xt[0:64, :, :],
                                    x_d[:, 16 * t:16 * t + 8, :])
                nc.sync.dma_start(xt[64:128, :, :],
                                  x_d[:, 16 * t + 8:16 * t + 16, :])
                x_tiles[t] = xt

            def feat(t):
                ft = fpsum.tile([128, 8, W], fp32, tag="ft")
                fs = fsbp.tile([128, 8, W], bf16, tag="fs")
                for half in range(2):
                    nc.tensor.matmul(ft[:, 4 * half:4 * half + 4, :],
                                     w1_sb[:],
                                     x_tiles[t][:, 4 * half:4 * half + 4, :],
                                     start=True, stop=True)
                    # bf16 copy feeding the DVE relu variants
                    nc.scalar.activation(fs[:, 4 * half:4 * half + 4, :],
                                         ft[:, 4 * half:4 * half + 4, :],
                                         mybir.ActivationFunctionType.Copy)
                f_tiles[t] = ft
                fs_tiles[t] = fs

            def relu4(t, r):
                """h = relu(F + c[r,s]) for the 4 s-variants of row-phase r."""
                ft, fs = f_tiles[t], fs_tiles[t]
                hr = hbuf.tile([128, 4, 8, W], bf16, tag="hr")
                for s in range(4):
                    v = 4 * r + s
                    bias = crs_sb[:, v:v + 1]
                    if RELU_ENG[r][s] == "act":
                        nc.scalar.activation(
                            hr[:, s], ft[:],
                            mybir.ActivationFunctionType.Relu, bias=bias)
                    else:
                        nc.vector.tensor_scalar(
                            hr[:, s], fs[:], bias, 0.0,
                            mybir.AluOpType.add, mybir.AluOpType.max)
                return hr

            def pred(t, r, hr, st):
                pt = ppsum.tile([102, 2, OUT], fp32, tag="pt")
                for b in range(4):
                    for j in range(2):
                        i = 2 * b + j
                        rhs = hr[:, :, i, :].rearrange("p s l -> p l s")
                        nc.tensor.matmul(pt[32 * b:32 * b + 6, j, :],
                                         w2_sb[:], rhs,
                                         start=True, stop=True,
                                         tile_position=(0, 32 * b))
                if STAGE_ENG[r] == "act":
                    nc.scalar.activation(st[0:102, r], pt[:],
                                         mybir.ActivationFunctionType.Copy)
                else:
                    nc.vector.tensor_copy(st[0:102, r], pt[:])

            def dump(t, st):
                for b in range(4):
                    nc.gpsimd.dma_start(
                        out_d[t, b, :, :, :],
                        st[32 * b:32 * b + 6].rearrange("p r j w -> p r (j w)"))

            # software pipeline: feat(t+1) is emitted mid-tile so its PE
            # slot comes after pred(t, r0/r1), by which time the x(t+1)
            # DMA (issued at tile start) has landed -- emitting it first
            # made the statically-ordered PE queue stall on the DMA.
            load_x(0)
            feat(0)
            for t in range(NT):
                if t + 1 < NT:
                    load_x(t + 1)
                hrs = [relu4(t, r) for r in range(4)]
                st = stagep.tile([128, 4, 2, OUT], bf16, tag="st")
                pred(t, 0, hrs[0], st)
                pred(t, 1, hrs[1], st)
                if t + 1 < NT:
                    feat(t + 1)
                pred(t, 2, hrs[2], st)
                pred(t, 3, hrs[3], st)
                dump(t, st)

    nc.compile()
    _CACHE["nc"] = nc
    return nc


def _prep_inputs(x, W1, b1, W2, b2):
    x = np.ascontiguousarray(np.asarray(x, dtype=np.float32))
    W1 = np.asarray(W1, dtype=np.float32)
    b1 = np.asarray(b1, dtype=np.float32)
    W2 = np.asarray(W2, dtype=np.float32)

    w1c = W1[:NF]                      # [64, 64]
    w1diag = np.zeros((128, 128), dtype=ml_dtypes.bfloat16)
    w1diag[0:64, 0:64] = w1c.astype(ml_dtypes.bfloat16)
    w1diag[64:128, 64:128] = w1c.astype(ml_dtypes.bfloat16)

    w2diag = np.zeros((128, 6), dtype=ml_dtypes.bfloat16)
    w2diag[0:64, 0:3] = W2.astype(ml_dtypes.bfloat16)
    w2diag[64:128, 3:6] = W2.astype(ml_dtypes.bfloat16)

    # c[r,s] = rel[r]*W1[64] + rel[s]*W1[65] + b1 -> [16, 64]
    crs = (REL[:, None, None] * W1[NF][None, None, :]
           + REL[None, :, None] * W1[NF + 1][None, None, :]
           + b1[None, None, :]).reshape(16, NF)
    crsT = np.ascontiguousarray(
        np.concatenate([crs.T, crs.T], axis=0)).astype(np.float32)  # [128,16]

    xb = x.astype(ml_dtypes.bfloat16)
    in_maps = []
    for c in range(N_CORES):
        b, half = c // 2, c % 2
        xs = np.ascontiguousarray(xb[b, :, half * RPC:(half + 1) * RPC, :])
        in_maps.append({"x": xs, "w1diag": w1diag, "w2diag": w2diag,
                        "crsT": crsT})
    return in_maps


def _gather(results, b2):
    full = np.empty((B, 3, OUT, OUT), dtype=np.float32)
    for c in range(N_CORES):
        b, half = c // 2, c % 2
        # raw dump [4t+r, p=32b'+3g+ch, (j, w)] -> rows 64t+32g+8b'+4j+r
        raw = np.asarray(results[c]["out"]).astype(np.float32)
        # [t, bq, 3g+ch, r, (j, w)] -> rows 64t + 32g + 8bq + 4j + r
        arr = raw.reshape(NT, 4, 2, 3, 4, 2, OUT)      # t,bq,g,ch,r,j,w
        oc = np.empty((3, OUT // 2, OUT), dtype=np.float32)
        ocv = oc.reshape(3, NT, 2, 4, 2, 4, OUT)       # ch,t,g,bq,j,r,w
        ocv[:] = arr.transpose(3, 0, 2, 1, 5, 4, 6)    # ch,t,g,bq,j,r,w
        full[b, :, half * (OUT // 2):(half + 1) * (OUT // 2), :] = oc
    b2 = np.asarray(b2, dtype=np.float32)
    if np.any(b2):
        full += b2.reshape(1, 3, 1, 1)
    return full


def run(trace=False, **inputs):
    nc = _build_program()
    in_maps = _prep_inputs(inputs["x"], inputs["W1"], inputs["b1"],
                           inputs["W2"], inputs["b2"])
    res = run_bass_kernel_spmd(nc, in_maps, list(range(N_CORES)), trace=trace)
    return _gather(res.results, inputs["b2"]), res


def kernel(**inputs):
    out, _ = run(trace=False, **inputs)
    return out
